# revision 1
# baseline (speedup 1.0000x reference)
"""Trainium2 Bass kernel for CMPGNN message passing (8-core SPMD).

Sharding: nodes split contiguously across 8 cores (graph parallel).
Per layer each core computes h3/h4/h1 for its own nodes, stages an
fp16 [G=h3 | B=-h4] table shard, and AllGathers the full table into
every core's DRAM. Edges are routed to the core that owns their TARGET
node, in "degree-aligned slots": edge (u->v) occupies SBUF partition
(v mod 128) of its target tile, so segment_sum collapses to one
strided vector reduce. Edges beyond the per-node slot budget go to
small one-hot-matmul overflow chunks. Source features come in via
GPSIMD dma_gather (int16 indices, so the table is addressed through a
"lo" window [0, 32768) and a "hi" window [Ntab-32768, Ntab), with each
edge slotted into the grid its source row is addressable from; a
natural always-zero pad row inside the window overlap backs empty
slots).
"""

import os
import sys
import math
import numpy as np

sys.path.insert(0, "/opt/trn_rl_repo")

from concourse import bass, bacc, mybir, tile  # noqa: E402
from concourse.masks import make_identity  # noqa: E402

AF = mybir.ActivationFunctionType
OP = mybir.AluOpType
DT = mybir.dt
AX = mybir.AxisListType

P = 128    # partitions == hidden size H
IW = 32768  # int16 index window


class Cfg:
    def __init__(self, N=50000, E=600000, F=500, H=128, C=40, KL=4, W=8,
                 iw=IW):
        assert H == P
        self.IW = iw
        self.N, self.E, self.F, self.H, self.C, self.KL, self.W = N, E, F, H, C, KL, W
        self.NL = N // W                       # owned nodes per core
        assert self.NL * W == N
        # >=1 pad node per core guaranteed (pad rows double as zero rows)
        self.T = math.ceil((self.NL + 1) / P)  # node tiles per core
        self.NLp = self.T * P                  # padded nodes per core
        self.Fp = math.ceil(F / P) * P         # padded input features
        self.FC = self.Fp // P                 # input feature chunks
        self.Ntab = self.NLp * W               # table rows
        assert self.Ntab <= 2 * self.IW - 1
        self.HIOFF = max(self.Ntab - self.IW, 0)    # hi window base (0 = no split)


def _wrap_idx16(flat):
    """dma_gather index layout: [i%16, i//16], replicated to 128 partitions."""
    n = flat.shape[0]
    assert n % 16 == 0
    blk = np.full((16, n // 16), -1, dtype=np.int16)
    blk[np.arange(n) % 16, np.arange(n) // 16] = flat.astype(np.int16)
    return np.tile(blk, (8, 1))


def plan(cfg, edge_index):
    """Host-side edge routing. Returns meta dict + per-core input arrays."""
    W, NL, NLp, T, HIOFF = cfg.W, cfg.NL, cfg.NLp, cfg.T, cfg.HIOFF
    row = np.asarray(edge_index[0], dtype=np.int64)
    col = np.asarray(edge_index[1], dtype=np.int64)

    kk = row // NL
    srow = kk * NLp + (row - kk * NL)          # source table row
    core_of = col // NL
    tile_of = (col - core_of * NL) // P
    part_of = (col - core_of * NL) % P
    grid_of = (srow >= cfg.IW).astype(np.int64) if HIOFF > 0 else np.zeros_like(srow)

    # zero row: a pad row visible from both index windows
    zrows = [k * NLp + NL for k in range(W) if HIOFF <= k * NLp + NL < cfg.IW]
    assert zrows, "no pad row in the lo/hi overlap window"
    zrow = zrows[0]

    meta = dict(D=[0, 0], OF_t=[[0] * T, [0] * T], zrow=zrow)
    per_core = [dict() for _ in range(W)]
    ngrids = 2 if HIOFF > 0 else 1

    for g in range(ngrids):
        gsel = grid_of == g
        deg = np.zeros((W, T, P), dtype=np.int64)
        np.add.at(deg, (core_of[gsel], tile_of[gsel], part_of[gsel]), 1)
        best = None
        # cap D at 8: one <=1024-index dma_gather per grid per tile
        for D in range(0 if g > 0 else 1, min(int(deg.max()), 8) + 1):
            of_cnt = np.maximum(deg - D, 0).sum(axis=2)
            OF_t = np.ceil(of_cnt / P).astype(np.int64).max(axis=0)
            cost = T * P * D + 4 * int(OF_t.sum()) * P
            if best is None or cost < best[0]:
                best = (cost, D, OF_t)
        _, D, OF_t = best
        OF_t = [int(v) for v in OF_t]
        meta["D"][g] = D
        meta["OF_t"][g] = OF_t
        OFsum = int(sum(OF_t))
        base = HIOFF if g == 1 else 0
        zloc = zrow - base

        order = np.lexsort((part_of, tile_of, core_of))
        osel = gsel[order]
        oo = order[osel]
        cs, ts, ps = core_of[oo], tile_of[oo], part_of[oo]
        sr, cl = srow[oo] - base, part_of[oo]
        of_base = np.concatenate([[0], np.cumsum(OF_t)]).astype(np.int64)

        for k in range(W):
            idx_base = np.full((T, P, D), zloc, dtype=np.int64)
            idx_of = np.full((max(OFsum, 1), P), zloc, dtype=np.int64)
            col_of = np.full((P, max(OFsum, 1)), -1.0, dtype=np.float16)
            m = cs == k
            kt, kp, kr = ts[m], ps[m], sr[m]
            i, n = 0, kt.shape[0]
            of_fill = np.zeros(T, dtype=np.int64)
            while i < n:
                t, p = kt[i], kp[i]
                j = i
                while j < n and kt[j] == t and kp[j] == p:
                    j += 1
                nb = min(j - i, D)
                idx_base[t, p, :nb] = kr[i: i + nb]
                for e in range(i + nb, j):
                    f = of_fill[t]
                    of_fill[t] += 1
                    oj, op_ = divmod(int(f), P)
                    idx_of[of_base[t] + oj, op_] = kr[e]
                    col_of[op_, of_base[t] + oj] = np.float16(p)
                i = j
            # dma_gather int16 wrapped layouts; flat order = (chunk, partition)
            if D > 0:
                fb = np.concatenate(
                    [idx_base[t].T.reshape(-1) for t in range(T)])
                per_core[k][f"ixb{g}"] = _wrap_idx16(fb)
            if OFsum > 0:
                fo = idx_of[:OFsum].reshape(-1)
                per_core[k][f"ixo{g}"] = _wrap_idx16(fo)
            per_core[k][f"co{g}"] = col_of
    return meta, per_core


def build(cfg, meta, dtype16=DT.float16):
    """Build + compile the SPMD Tile kernel."""
    W, T, NLp, C, KL, FC = cfg.W, cfg.T, cfg.NLp, cfg.C, cfg.KL, cfg.FC
    f32 = DT.float32
    DL, DH = meta["D"]
    Dt = DL + DH
    OFL_t, OFH_t = meta["OF_t"]
    OFLs, OFHs = int(sum(OFL_t)), int(sum(OFH_t))
    ofl_base = np.concatenate([[0], np.cumsum(OFL_t)]).astype(np.int64)
    ofh_base = np.concatenate([[0], np.cumsum(OFH_t)]).astype(np.int64)

    nc = bacc.Bacc("TRN2", target_bir_lowering=False, debug=False,
                   num_devices=W)

    xT = nc.dram_tensor("xT", [cfg.Fp, NLp], dtype16, kind="ExternalInput")
    WinT = nc.dram_tensor("WinT", [cfg.Fp, P], dtype16, kind="ExternalInput")
    W1T = nc.dram_tensor("W1T", [KL, P, P], dtype16, kind="ExternalInput")
    W2T = nc.dram_tensor("W2T", [KL, P, P], dtype16, kind="ExternalInput")
    WfT = nc.dram_tensor("WfT", [KL, P, P], dtype16, kind="ExternalInput")
    WoutT = nc.dram_tensor("WoutT", [P, C], dtype16, kind="ExternalInput")
    b_in_rep = nc.dram_tensor("b_in_rep", [P, P], f32, kind="ExternalInput")
    b_in_lst = nc.dram_tensor("b_in_lst", [P, P], f32, kind="ExternalInput")
    b_out_rep = nc.dram_tensor("b_out_rep", [P, C], f32, kind="ExternalInput")
    iota_rep = nc.dram_tensor("iota_rep", [P, P], dtype16, kind="ExternalInput")
    ins16 = {}
    for name, cols in [("ixb0", T * 8 * DL), ("ixb1", T * 8 * DH),
                       ("ixo0", 8 * OFLs), ("ixo1", 8 * OFHs)]:
        if cols > 0:
            ins16[name] = nc.dram_tensor(name, [P, cols], DT.int16,
                                         kind="ExternalInput")
    co0 = nc.dram_tensor("co0", [P, max(OFLs, 1)], dtype16, kind="ExternalInput")
    co1 = nc.dram_tensor("co1", [P, max(OFHs, 1)], dtype16, kind="ExternalInput")
    out = nc.dram_tensor("out", [NLp, C], f32, kind="ExternalOutput")

    agin = nc.dram_tensor("agin", [NLp, 256], dtype16)
    table = nc.dram_tensor("table", [cfg.Ntab, 256], dtype16,
                           addr_space="Shared" if W > 4 else "Local")

    agin_v = agin.ap().rearrange("(t p) f -> p t f", p=P)
    out_v = out.ap().rearrange("(t p) c -> p t c", p=P)
    npad = NLp - cfg.NL   # pad nodes (tail partitions of last tile)

    with tile.TileContext(nc) as tc:
        with (
            tc.tile_pool(name="persist", bufs=1) as pp,
            tc.tile_pool(name="small", bufs=2) as mp,
            tc.tile_pool(name="psum", bufs=2, space="PSUM") as psp,
            tc.tile_pool(name="psacc", bufs=1, space="PSUM") as ps1,
            tc.tile_pool(name="ohp",
                         bufs=2 * max(2, max((a + b) for a, b in
                                             zip(OFL_t, OFH_t)))) as ohp,
        ):
            QT = pp.tile([P, T * P], dtype16, tag="QT")
            h1 = pp.tile([P, T * P], f32, tag="h1")       # also hq^2 scratch
            UVs = pp.tile([P, T * 256], dtype16, tag="UVs")
            hq = pp.tile([P, T * P], f32, tag="hq")       # node feats (row-major)
            ss = pp.tile([P, T], f32, tag="ss")
            rinv = pp.tile([P, T], f32, tag="rinv")
            ident = pp.tile([P, P], f32, tag="ident")
            ident16 = pp.tile([P, P], dtype16, tag="ident16")
            make_identity(nc, ident[:])
            make_identity(nc, ident16[:])

            sb16 = {}
            for name, t_ in ins16.items():
                sb16[name] = pp.tile([P, t_.shape[1]], DT.int16, tag=name, name=f"sb_{name}")
                nc.sync.dma_start(out=sb16[name][:], in_=t_[:, :])
            co0s = pp.tile([P, max(OFLs, 1)], dtype16, tag="co0s")
            co1s = pp.tile([P, max(OFHs, 1)], dtype16, tag="co1s")
            nc.sync.dma_start(out=co0s[:], in_=co0[:, :])
            nc.sync.dma_start(out=co1s[:], in_=co1[:, :])
            iota = pp.tile([P, P], dtype16, tag="iota")
            binr = pp.tile([P, P], f32, tag="binr")
            binl = pp.tile([P, P], f32, tag="binl")
            boutr = pp.tile([P, C], f32, tag="boutr")
            w1 = pp.tile([P, KL * P], dtype16, tag="w1")
            w2 = pp.tile([P, KL * P], dtype16, tag="w2")
            wf = pp.tile([P, KL * P], dtype16, tag="wf")
            wo = pp.tile([P, C], dtype16, tag="wo")
            winT = pp.tile([P, FC * P], dtype16, tag="winT")
            ostage = pp.tile([P, T * C], f32, tag="ostage")

            nc.sync.dma_start(out=iota[:], in_=iota_rep[:, :])
            nc.sync.dma_start(out=binr[:], in_=b_in_rep[:, :])
            nc.sync.dma_start(out=binl[:], in_=b_in_lst[:, :])
            nc.sync.dma_start(out=boutr[:], in_=b_out_rep[:, :])
            nc.sync.dma_start(out=wo[:], in_=WoutT[:, :])
            for l in range(KL):
                nc.sync.dma_start(out=w1[:, l * P:(l + 1) * P], in_=W1T[l, :, :])
                nc.sync.dma_start(out=w2[:, l * P:(l + 1) * P], in_=W2T[l, :, :])
                nc.sync.dma_start(out=wf[:, l * P:(l + 1) * P], in_=WfT[l, :, :])
            nc.sync.dma_start(
                out=winT[:].rearrange("p (c q) -> p c q", c=FC),
                in_=WinT.ap().rearrange("(c p) q -> p c q", p=P))

            # registers for dma_gather num_idxs
            regs = {}
            for d_ in (DL, DH, *OFL_t, *OFH_t):
                for s0 in range(0, d_, 8):
                    n_ = P * (min(s0 + 8, d_) - s0)
                    if n_ > 0 and n_ not in regs:
                        regs[n_] = nc.gpsimd.to_reg(n_)

            # ---- phase 0: Q0 = x @ Win.T + b_in (into hq) ----
            with tc.tile_pool(name="ph0", bufs=1) as p0:
                xall = p0.tile([P, FC * NLp], dtype16, tag="xall")
                nc.sync.dma_start(
                    out=xall[:].rearrange("p (c n) -> p c n", c=FC),
                    in_=xT.ap().rearrange("(c p) n -> p c n", p=P))
                for t in range(T):
                    q0p = psp.tile([P, P], f32, space="PSUM", tag="tp")
                    for c in range(FC):
                        nc.tensor.matmul(
                            out=q0p[:],
                            lhsT=xall[:, c * NLp + t * P: c * NLp + (t + 1) * P],
                            rhs=winT[:, c * P:(c + 1) * P],
                            start=(c == 0), stop=(c == FC - 1))
                    nc.vector.tensor_tensor(
                        out=hq[:, t * P:(t + 1) * P], in0=q0p[:],
                        in1=(binr[:] if t < T - 1 else binl[:]), op=OP.add)

            GMAX = 8  # dma_gather hangs above 1024 indices per call

            def seg_gather(dst3, dbase, src_ap, ixs, colbase, D_):
                for s0 in range(0, D_, GMAX):
                    s1 = min(s0 + GMAX, D_)
                    nc.gpsimd.dma_gather(
                        dst3[:, dbase + s0:dbase + s1, :], src_ap,
                        ixs[:, colbase + s0 * 8: colbase + s1 * 8],
                        P * (s1 - s0), regs[P * (s1 - s0)], 256)

            def gathers(sp, t):
                """Issue base gathers for tile t, return gathered [P,Dt,256]."""
                g = sp.tile([P, Dt * 256], dtype16, tag="g")
                g3 = g[:].rearrange("p (d f) -> p d f", d=Dt)
                if DL > 0:
                    seg_gather(g3, 0, table.ap(), sb16["ixb0"], t * 8 * DL, DL)
                if DH > 0:
                    seg_gather(g3, DL, table.ap()[cfg.HIOFF:cfg.Ntab, :],
                               sb16["ixb1"], t * 8 * DH, DH)
                return g, g3

            def of_block(sp, t, vt, hqt):
                """Overflow chunks for tile t (both grids)."""
                OFL, OFH = OFL_t[t], OFH_t[t]
                OF = OFL + OFH
                if OF == 0:
                    return
                go = sp.tile([P, OF * 256], dtype16, tag="go")
                go3 = go[:].rearrange("p (d f) -> p d f", d=OF)
                if OFL > 0:
                    seg_gather(go3, 0, table.ap(), sb16["ixo0"],
                               int(ofl_base[t]) * 8, OFL)
                if OFH > 0:
                    seg_gather(go3, OFL, table.ap()[cfg.HIOFF:cfg.Ntab, :],
                               sb16["ixo1"], int(ofh_base[t]) * 8, OFH)
                go4 = go[:].rearrange("p (d h f) -> p d h f", d=OF, h=2)
                dof = mp.tile([P, OF], f32, tag="dof")
                ohs = []
                for oj in range(OF):
                    if oj < OFL:
                        cslice = co0s[:, int(ofl_base[t]) + oj:
                                      int(ofl_base[t]) + oj + 1]
                    else:
                        cslice = co1s[:, int(ofh_base[t]) + (oj - OFL):
                                      int(ofh_base[t]) + (oj - OFL) + 1]
                    oh = ohp.tile([P, P], dtype16, tag="oh")
                    nc.vector.tensor_tensor(
                        out=oh[:], in0=cslice.to_broadcast([P, P]),
                        in1=iota[:], op=OP.is_equal)
                    ohs.append(oh)
                    # expansion: -h4[col] rows for this chunk's slots
                    ohT = psp.tile([P, P], dtype16, space="PSUM", tag="pf")
                    nc.tensor.transpose(out=ohT[:], in_=oh[:],
                                        identity=ident16[:])
                    ohT16 = mp.tile([P, P], dtype16, tag="ohT16")
                    nc.scalar.activation(out=ohT16[:], in_=ohT[:], func=AF.Copy)
                    h4e = psp.tile([P, P], f32, space="PSUM", tag="p4")
                    nc.tensor.matmul(out=h4e[:], lhsT=ohT16[:], rhs=vt,
                                     start=True, stop=True)
                    pmo = mp.tile([P, P], dtype16, tag="pmo")
                    nc.vector.tensor_tensor(out=pmo[:], in0=go4[:, oj, 0, :],
                                            in1=h4e[:], op=OP.mult)
                    nc.vector.tensor_reduce(out=dof[:, oj:oj + 1], in_=pmo[:],
                                            axis=AX.X, op=OP.add)
                sao = mp.tile([P, OF], dtype16, tag="sao")
                sbo = mp.tile([P, OF], dtype16, tag="sbo")
                nc.scalar.activation(out=sao[:], in_=dof[:], func=AF.Sigmoid)
                nc.scalar.activation(out=sbo[:], in_=dof[:], func=AF.Sigmoid,
                                     scale=-1.0)
                wao = sp.tile([P, OF * P], dtype16, tag="wao")
                wbo = sp.tile([P, OF * P], dtype16, tag="wbo")
                wao3 = wao[:].rearrange("p (d f) -> p d f", d=OF)
                wbo3 = wbo[:].rearrange("p (d f) -> p d f", d=OF)
                sao3 = sao[:].rearrange("p (d o) -> p d o", o=1)
                sbo3 = sbo[:].rearrange("p (d o) -> p d o", o=1)
                nc.vector.tensor_tensor(
                    out=wao3, in0=go4[:, 0:OF, 0, :],
                    in1=sao3.to_broadcast([P, OF, P]), op=OP.mult)
                nc.vector.tensor_tensor(
                    out=wbo3, in0=go4[:, 0:OF, 1, :],
                    in1=sbo3.to_broadcast([P, OF, P]), op=OP.mult)
                aggo = ps1.tile([P, P], f32, space="PSUM", tag="aggo")
                for oj in range(OF):
                    nc.tensor.matmul(out=aggo[:], lhsT=ohs[oj][:],
                                     rhs=wao[:, oj * P:(oj + 1) * P],
                                     start=(oj == 0), stop=False)
                    nc.tensor.matmul(out=aggo[:], lhsT=ohs[oj][:],
                                     rhs=wbo[:, oj * P:(oj + 1) * P],
                                     start=False, stop=(oj == OF - 1))
                nc.vector.tensor_tensor(out=hqt, in0=hqt, in1=aggo[:],
                                        op=OP.add)

            with tc.tile_pool(name="stream", bufs=3) as sp:
                for li in range(KL):
                    # ---- phase 1 ----
                    for t in range(T):
                        tp = psp.tile([P, P], f32, space="PSUM", tag="tp")
                        nc.tensor.transpose(
                            out=tp[:], in_=hq[:, t * P:(t + 1) * P],
                            identity=ident[:])
                        nc.scalar.activation(
                            out=QT[:, t * P:(t + 1) * P], in_=tp[:], func=AF.Copy)
                    for t in range(T):
                        qt = QT[:, t * P:(t + 1) * P]
                        p3 = psp.tile([P, P], f32, space="PSUM", tag="tp")
                        p4 = psp.tile([P, P], f32, space="PSUM", tag="p4")
                        pf = psp.tile([P, P], f32, space="PSUM", tag="pf")
                        nc.tensor.matmul(out=p3[:], lhsT=qt,
                                         rhs=w1[:, li * P:(li + 1) * P],
                                         start=True, stop=True)
                        nc.tensor.matmul(out=p4[:], lhsT=qt,
                                         rhs=w2[:, li * P:(li + 1) * P],
                                         start=True, stop=True)
                        nc.tensor.matmul(out=pf[:], lhsT=qt,
                                         rhs=wf[:, li * P:(li + 1) * P],
                                         start=True, stop=True)
                        nc.scalar.activation(out=UVs[:, t * 256: t * 256 + P],
                                             in_=p3[:], func=AF.Copy)
                        nc.scalar.activation(out=UVs[:, t * 256 + P: t * 256 + 256],
                                             in_=p4[:], func=AF.Copy, scale=-1.0)
                        nc.scalar.activation(out=h1[:, t * P:(t + 1) * P],
                                             in_=pf[:], func=AF.Relu)
                    nc.sync.dma_start(
                        out=agin_v[:, :, :],
                        in_=UVs[:].rearrange("p (t f) -> p t f", t=T))
                    nc.gpsimd.collective_compute(
                        "AllGather", OP.bypass,
                        replica_groups=[list(range(W))],
                        ins=[agin.ap()], outs=[table.ap()])

                    # ---- phase 2: edges ----
                    for t in range(T):
                        g, g3 = gathers(sp, t)
                        g4 = g[:].rearrange("p (d h f) -> p d h f", d=Dt, h=2)
                        G, B = g4[:, :, 0, :], g4[:, :, 1, :]
                        vt = UVs[:, t * 256 + P: t * 256 + 256]
                        vt3 = vt.rearrange("p (o f) -> p o f", o=1)

                        pm = sp.tile([P, Dt * P], dtype16, tag="pm")
                        pm3 = pm[:].rearrange("p (d f) -> p d f", d=Dt)
                        nc.vector.tensor_tensor(
                            out=pm3, in0=G,
                            in1=vt3.to_broadcast([P, Dt, P]), op=OP.mult)
                        dall = mp.tile([P, Dt], f32, tag="dall")
                        nc.vector.tensor_reduce(
                            out=dall[:], in_=pm3, axis=AX.X, op=OP.add)
                        sa = mp.tile([P, Dt], dtype16, tag="sa")
                        sb = mp.tile([P, Dt], dtype16, tag="sb")
                        nc.scalar.activation(out=sa[:], in_=dall[:], func=AF.Sigmoid)
                        nc.scalar.activation(out=sb[:], in_=dall[:],
                                             func=AF.Sigmoid, scale=-1.0)
                        wab = sp.tile([P, Dt * 256], dtype16, tag="wab")
                        wab4 = wab[:].rearrange("p (d h f) -> p d h f", d=Dt, h=2)
                        sa3 = sa[:].rearrange("p (d o) -> p d o", o=1)
                        sb3 = sb[:].rearrange("p (d o) -> p d o", o=1)
                        nc.vector.tensor_tensor(
                            out=wab4[:, :, 0, :], in0=G,
                            in1=sa3.to_broadcast([P, Dt, P]), op=OP.mult)
                        nc.vector.tensor_tensor(
                            out=wab4[:, :, 1, :], in0=B,
                            in1=sb3.to_broadcast([P, Dt, P]), op=OP.mult)
                        agg = mp.tile([P, P], f32, tag="agg")
                        nc.vector.tensor_reduce(
                            out=agg[:],
                            in_=wab[:].rearrange("p (dh f) -> p f dh", f=P),
                            axis=AX.X, op=OP.add)
                        hqt = hq[:, t * P:(t + 1) * P]
                        nc.vector.tensor_tensor(
                            out=hqt, in0=h1[:, t * P:(t + 1) * P],
                            in1=agg[:], op=OP.add)
                        of_block(sp, t, vt, hqt)

                    # ---- batched L2 normalize (hq in place) ----
                    nc.vector.tensor_tensor(out=h1[:], in0=hq[:], in1=hq[:],
                                            op=OP.mult)
                    nc.vector.tensor_reduce(
                        out=ss[:], in_=h1[:].rearrange("p (t f) -> p t f", t=T),
                        axis=AX.X, op=OP.add)
                    nc.scalar.activation(out=ss[:], in_=ss[:], func=AF.Sqrt)
                    nc.vector.tensor_scalar_max(out=ss[:], in0=ss[:],
                                                scalar1=1e-12)
                    nc.vector.reciprocal(out=rinv[:], in_=ss[:])
                    rinv3 = rinv[:].rearrange("p (t o) -> p t o", o=1)
                    nc.vector.tensor_tensor(
                        out=hq[:].rearrange("p (t f) -> p t f", t=T),
                        in0=hq[:].rearrange("p (t f) -> p t f", t=T),
                        in1=rinv3.to_broadcast([P, T, P]), op=OP.mult)

                # ---- phase 3: logits + log_softmax ----
                for t in range(T):
                    tp = psp.tile([P, P], f32, space="PSUM", tag="tp")
                    nc.tensor.transpose(out=tp[:], in_=hq[:, t * P:(t + 1) * P],
                                        identity=ident[:])
                    nc.scalar.activation(out=QT[:, t * P:(t + 1) * P],
                                         in_=tp[:], func=AF.Copy)
                    lp = psp.tile([P, C], f32, space="PSUM", tag="p4")
                    nc.tensor.matmul(out=lp[:], lhsT=QT[:, t * P:(t + 1) * P],
                                     rhs=wo[:], start=True, stop=True)
                    lg = mp.tile([P, C], f32, tag="lg")
                    nc.vector.tensor_tensor(out=lg[:], in0=lp[:], in1=boutr[:],
                                            op=OP.add)
                    mx = mp.tile([P, 1], f32, tag="mx")
                    nc.vector.tensor_reduce(out=mx[:], in_=lg[:], axis=AX.X,
                                            op=OP.max)
                    nmx = mp.tile([P, 1], f32, tag="nmx")
                    nc.vector.tensor_scalar_mul(out=nmx[:], in0=mx[:],
                                                scalar1=-1.0)
                    ex = mp.tile([P, C], f32, tag="ex")
                    se = mp.tile([P, 1], f32, tag="se")
                    nc.scalar.activation(out=ex[:], in_=lg[:], func=AF.Exp,
                                         bias=nmx[:], accum_out=se[:])
                    nc.scalar.activation(out=se[:], in_=se[:], func=AF.Ln)
                    nc.vector.tensor_tensor(out=mx[:], in0=mx[:], in1=se[:],
                                            op=OP.add)
                    nc.vector.tensor_scalar(
                        out=ostage[:, t * C:(t + 1) * C], in0=lg[:],
                        scalar1=mx[:], scalar2=None, op0=OP.subtract)
                nc.sync.dma_start(
                    out=out_v[:, :, :],
                    in_=ostage[:].rearrange("p (t c) -> p t c", t=T))

    nc.compile()
    return nc


def host_inputs(cfg, meta, per_core, inputs, np16=np.float16):
    x = np.asarray(inputs["x"], np.float32)
    W, NL = cfg.W, cfg.NL
    iota = np.tile(np.arange(P, dtype=np16)[None, :], (P, 1))
    WinT = np.zeros((cfg.Fp, P), np16)
    WinT[:cfg.F] = np.asarray(inputs["W_in"], np.float32).T.astype(np16)
    W1T = np.ascontiguousarray(
        np.asarray(inputs["W1"], np.float32).transpose(0, 2, 1)).astype(np16)
    W2T = np.ascontiguousarray(
        np.asarray(inputs["W2"], np.float32).transpose(0, 2, 1)).astype(np16)
    WfT = np.ascontiguousarray(
        np.asarray(inputs["Wf"], np.float32).transpose(0, 2, 1)).astype(np16)
    WoT = np.ascontiguousarray(
        np.asarray(inputs["W_out"], np.float32).T).astype(np16)
    binr = np.tile(np.asarray(inputs["b_in"], np.float32)[None, :], (P, 1))
    boutr = np.tile(np.asarray(inputs["b_out"], np.float32)[None, :], (P, 1))
    maps = []
    for k in range(W):
        xk = np.zeros((cfg.Fp, cfg.NLp), np16)
        xk[:cfg.F, :NL] = x[k * NL:(k + 1) * NL].T.astype(np16)
        binl = binr.copy()
        binl[NL % P if NL % P else 0:] = 0.0   # pad-node rows of last tile
        m = dict(
            xT=xk, WinT=WinT, W1T=W1T, W2T=W2T, WfT=WfT, WoutT=WoT,
            b_in_rep=np.ascontiguousarray(binr, np.float32),
            b_in_lst=np.ascontiguousarray(binl, np.float32),
            b_out_rep=np.ascontiguousarray(boutr, np.float32),
            iota_rep=iota,
            co0=per_core[k].get("co0", np.full((P, 1), -1.0, np16)).astype(np16),
            co1=per_core[k].get("co1", np.full((P, 1), -1.0, np16)).astype(np16))
        for name in ("ixb0", "ixb1", "ixo0", "ixo1"):
            if name in per_core[k]:
                m[name] = per_core[k][name]
        maps.append(m)
    return maps


def _install_profile_hook():
    """Provide antenv.axon_hooks (absent in this image) so that
    run_bass_kernel_spmd(trace=True) can collect an NTFF profile."""
    try:
        import types
        import antenv
        if "antenv.axon_hooks" not in sys.modules:
            mod = types.ModuleType("antenv.axon_hooks")
            state = {"hook": None}
            mod.set_axon_ntff_profile_hook = lambda h: state.__setitem__("hook", h)
            mod.get_axon_ntff_profile_hook = lambda: state["hook"]
            sys.modules["antenv.axon_hooks"] = mod
            antenv.axon_hooks = mod
        from antenv.axon_hooks import (get_axon_ntff_profile_hook,
                                       set_axon_ntff_profile_hook)
        if get_axon_ntff_profile_hook() is None:
            from trn_agent_boot.trn_boot import _ntff_profile_via_ctypes
            set_axon_ntff_profile_hook(
                _ntff_profile_via_ctypes("/opt/axon/libaxon_pjrt.so"))
        return True
    except Exception as e:  # degrade to untraced run
        print(f"profile hook unavailable: {e}")
        return False


def kernel(**inputs):
    cfg = Cfg()
    edge_index = np.asarray(inputs["edge_index"])
    meta, per_core = plan(cfg, edge_index)
    nc = build(cfg, meta)
    in_maps = host_inputs(cfg, meta, per_core, inputs)
    trace = bool(int(os.environ.get("GNN_TRACE", "0")))
    if trace:
        trace = _install_profile_hook()
    from concourse import bass_utils
    res = bass_utils.run_bass_kernel_spmd(
        nc, in_maps, core_ids=list(range(cfg.W)), trace=trace)
    if res.exec_time_ns is not None:
        print(f"HW exec time: {res.exec_time_ns} ns")
    outs = [res.results[k]["out"][:cfg.NL] for k in range(cfg.W)]
    return np.concatenate(outs, axis=0).astype(np.float32)



# revision 4
# speedup vs baseline: 1.1721x; 1.1721x over previous
"""Trainium2 Bass kernel for CMPGNN message passing (8-core SPMD), v2.

Sharding: nodes split contiguously across 8 cores (graph parallel).
Per layer each core computes h3/h4/h1 for its own nodes, publishes an
fp16 [G=h3 | W=h3+h4] table shard, and AllGathers the full table.

Edge phase ("edge blocks"): per target tile (128 target nodes) the
core's incoming edges are packed densely into 128-edge blocks (split by
int16 index window lo/hi, padded per window). One dma_gather per
(tile, window) fetches the source rows edge-major: gather chunk b =
edge block b, edge j on partition j%128. Per block:
    hsel = OH1^T @ H4tile          (TensorE, one-hot select h4[v_e])
    d    = rowsum(G * hsel)        (DVE tensor_tensor_reduce)
    q    = sigmoid(d)              (ACT)
    m    = q*W - G                 (DVE scalar_tensor_tensor)
    agg += OH2^T @ m               (TensorE, OH2 = -one-hot, accumulates
                                    sum(s*W - h4) over the tile's blocks)
OH1/OH2 are compile-time constants streamed from DRAM per tile.
Dummy (padding) edges gather a guaranteed-zero table row and have zero
OH1/OH2 columns/rows, so they contribute nothing.
"""

import os
import sys
import math
import numpy as np

sys.path.insert(0, "/opt/trn_rl_repo")

from concourse import bass, bacc, mybir, tile  # noqa: E402
from concourse.masks import make_identity  # noqa: E402

AF = mybir.ActivationFunctionType
OP = mybir.AluOpType
DT = mybir.dt
AX = mybir.AxisListType

P = 128    # partitions == hidden size H
IW = 32768  # int16 index window


class Cfg:
    def __init__(self, N=50000, E=600000, F=500, H=128, C=40, KL=4, W=8):
        assert H == P
        self.N, self.E, self.F, self.H, self.C, self.KL, self.W = N, E, F, H, C, KL, W
        self.NL = N // W                       # owned nodes per core
        assert self.NL * W == N
        # >=1 pad node per core guaranteed (pad rows double as zero rows)
        self.T = math.ceil((self.NL + 1) / P)  # node tiles per core
        self.NLp = self.T * P                  # padded nodes per core
        self.Fp = math.ceil(F / P) * P         # padded input features
        self.FC = self.Fp // P                 # input feature chunks
        self.Ntab = self.NLp * W               # table rows
        assert self.Ntab <= 2 * IW - 1
        self.HIOFF = max(self.Ntab - IW, 0)    # hi window base


def _wrap_idx16(flat):
    """dma_gather index layout: [i%16, i//16], replicated to 128 partitions."""
    n = flat.shape[0]
    assert n % 16 == 0
    blk = np.full((16, n // 16), -1, dtype=np.int16)
    blk[np.arange(n) % 16, np.arange(n) // 16] = flat.astype(np.int16)
    return np.tile(blk, (8, 1))


def plan(cfg, edge_index):
    """Host-side edge routing into per-(tile,window) 128-edge blocks."""
    W, NL, NLp, T, HIOFF = cfg.W, cfg.NL, cfg.NLp, cfg.T, cfg.HIOFF
    row = np.asarray(edge_index[0], dtype=np.int64)
    col = np.asarray(edge_index[1], dtype=np.int64)
    kk = row // NL
    srow = kk * NLp + (row - kk * NL)          # global table row of source
    core_of = col // NL
    tile_of = (col - core_of * NL) // P
    part_of = (col - core_of * NL) % P

    zrow = next(k * NLp + NL for k in range(W) if HIOFF <= k * NLp + NL < IW)

    # window categories: 0 lo-only, 1 flex (overlap), 2 hi-only
    cat = np.where(srow < HIOFF, 0, np.where(srow < IW, 1, 2))

    nb_lo = np.zeros((W, T), dtype=np.int64)
    nb_hi = np.zeros((W, T), dtype=np.int64)
    edat = {}
    for k in range(W):
        mk = core_of == k
        for t in range(T):
            eids = np.nonzero(mk & (tile_of == t))[0]
            c = cat[eids]
            lo0, fl, hi0 = eids[c == 0], eids[c == 1], eids[c == 2]
            best = None
            nf = fl.shape[0]
            for a in range(nf + 1):
                blo = (lo0.shape[0] + a + P - 1) // P
                bhi = (hi0.shape[0] + (nf - a) + P - 1) // P
                key = (blo + bhi, abs((lo0.shape[0] + a) - (hi0.shape[0] + nf - a)))
                if best is None or key < best[0]:
                    best = (key, a)
            a = best[1]
            elo = np.concatenate([lo0, fl[:a]])
            ehi = np.concatenate([hi0, fl[a:]])
            nb_lo[k, t] = (elo.shape[0] + P - 1) // P
            nb_hi[k, t] = (ehi.shape[0] + P - 1) // P
            edat[(k, t)] = (elo, ehi)

    NB_lo = [int(x) for x in nb_lo.max(axis=0)]
    NB_hi = [int(x) for x in nb_hi.max(axis=0)]
    NBtot = int(sum(NB_lo) + sum(NB_hi))
    meta = dict(NB_lo=NB_lo, NB_hi=NB_hi, NBtot=NBtot, zrow=zrow)

    per_core = []
    for k in range(W):
        ixlo_l, ixhi_l = [], []
        oh1 = np.zeros((P, NBtot * P), dtype=np.float16)
        oh2 = np.zeros((P, NBtot * P), dtype=np.float16)
        b0 = 0
        for t in range(T):
            elo, ehi = edat[(k, t)]
            slo = np.full(NB_lo[t] * P, zrow, dtype=np.int64)
            slo[:elo.shape[0]] = srow[elo]
            shi = np.full(NB_hi[t] * P, zrow - HIOFF, dtype=np.int64)
            shi[:ehi.shape[0]] = srow[ehi] - HIOFF
            ixlo_l.append(slo)
            ixhi_l.append(shi)
            for off, elist in ((0, elo), (NB_lo[t] * P, ehi)):
                for j, e in enumerate(elist):
                    sl = off + j
                    b = b0 + sl // P
                    oh1[part_of[e], b * P + sl % P] = 1.0
                    oh2[sl % P, b * P + part_of[e]] = -1.0
            b0 += NB_lo[t] + NB_hi[t]
        per_core.append(dict(
            ixlo=_wrap_idx16(np.concatenate(ixlo_l)),
            ixhi=_wrap_idx16(np.concatenate(ixhi_l)),
            oh1=oh1, oh2=oh2))
    return meta, per_core


def build(cfg, meta, dtype16=DT.float16):
    """Build + compile the SPMD Tile kernel."""
    W, T, NLp, C, KL, FC = cfg.W, cfg.T, cfg.NLp, cfg.C, cfg.KL, cfg.FC
    f32 = DT.float32
    NB_lo, NB_hi = meta["NB_lo"], meta["NB_hi"]
    NB_t = [a + b for a, b in zip(NB_lo, NB_hi)]
    NBtot = meta["NBtot"]
    NBLO, NBHI = sum(NB_lo), sum(NB_hi)
    NBmax = max(NB_t)

    nc = bacc.Bacc("TRN2", target_bir_lowering=False, debug=False,
                   num_devices=W)

    xT = nc.dram_tensor("xT", [cfg.Fp, NLp], dtype16, kind="ExternalInput")
    WinT = nc.dram_tensor("WinT", [cfg.Fp, P], dtype16, kind="ExternalInput")
    W1T = nc.dram_tensor("W1T", [KL, P, P], dtype16, kind="ExternalInput")
    W2T = nc.dram_tensor("W2T", [KL, P, P], dtype16, kind="ExternalInput")
    WfT = nc.dram_tensor("WfT", [KL, P, P], dtype16, kind="ExternalInput")
    WoutT = nc.dram_tensor("WoutT", [P, C], dtype16, kind="ExternalInput")
    b_in_rep = nc.dram_tensor("b_in_rep", [P, P], f32, kind="ExternalInput")
    b_in_lst = nc.dram_tensor("b_in_lst", [P, P], f32, kind="ExternalInput")
    b_out_rep = nc.dram_tensor("b_out_rep", [P, C], f32, kind="ExternalInput")
    ixlo = nc.dram_tensor("ixlo", [P, NBLO * 8], DT.int16, kind="ExternalInput")
    ixhi = nc.dram_tensor("ixhi", [P, NBHI * 8], DT.int16, kind="ExternalInput")
    oh1 = nc.dram_tensor("oh1", [P, NBtot * P], dtype16, kind="ExternalInput")
    oh2 = nc.dram_tensor("oh2", [P, NBtot * P], dtype16, kind="ExternalInput")
    out = nc.dram_tensor("out", [NLp, C], f32, kind="ExternalOutput")

    agin = nc.dram_tensor("agin", [NLp, 256], dtype16)
    table = nc.dram_tensor("table", [cfg.Ntab, 256], dtype16,
                           addr_space="Shared" if W > 4 else "Local")

    agin_v = agin.ap().rearrange("(t p) f -> p t f", p=P)
    out_v = out.ap().rearrange("(t p) c -> p t c", p=P)

    with tile.TileContext(nc) as tc:
        with (
            tc.tile_pool(name="persist", bufs=1) as pp,
            tc.tile_pool(name="small", bufs=6) as mp,
            tc.tile_pool(name="psum", bufs=1, space="PSUM") as psp,
            tc.tile_pool(name="hselp", bufs=3, space="PSUM") as hp,
            tc.tile_pool(name="aggp", bufs=2, space="PSUM") as ap_,
        ):
            QT = pp.tile([P, T * P], dtype16, tag="QT")
            h1 = pp.tile([P, T * P], dtype16, tag="h1")
            h4sb = pp.tile([P, T * P], dtype16, tag="h4sb")
            uv = pp.tile([P, T * 256], dtype16, tag="uv")   # [G|W] staging
            hq = pp.tile([P, T * P], f32, tag="hq")
            sq = pp.tile([P, T * P], f32, tag="sq")         # norm scratch
            ss = pp.tile([P, T], f32, tag="ss")
            rinv = pp.tile([P, T], f32, tag="rinv")
            ident = pp.tile([P, P], f32, tag="ident")
            make_identity(nc, ident[:])

            sxlo = pp.tile([P, NBLO * 8], DT.int16, tag="sxlo")
            sxhi = pp.tile([P, NBHI * 8], DT.int16, tag="sxhi")
            nc.sync.dma_start(out=sxlo[:], in_=ixlo[:, :])
            nc.sync.dma_start(out=sxhi[:], in_=ixhi[:, :])

            binr = pp.tile([P, P], f32, tag="binr")
            binl = pp.tile([P, P], f32, tag="binl")
            boutr = pp.tile([P, C], f32, tag="boutr")
            w1 = pp.tile([P, KL * P], dtype16, tag="w1")
            w2 = pp.tile([P, KL * P], dtype16, tag="w2")
            wf = pp.tile([P, KL * P], dtype16, tag="wf")
            wo = pp.tile([P, C], dtype16, tag="wo")
            winT = pp.tile([P, FC * P], dtype16, tag="winT")
            ostage = pp.tile([P, T * C], f32, tag="ostage")

            nc.sync.dma_start(out=binr[:], in_=b_in_rep[:, :])
            nc.sync.dma_start(out=binl[:], in_=b_in_lst[:, :])
            nc.sync.dma_start(out=boutr[:], in_=b_out_rep[:, :])
            nc.sync.dma_start(out=wo[:], in_=WoutT[:, :])
            for l in range(KL):
                nc.sync.dma_start(out=w1[:, l * P:(l + 1) * P], in_=W1T[l, :, :])
                nc.sync.dma_start(out=w2[:, l * P:(l + 1) * P], in_=W2T[l, :, :])
                nc.sync.dma_start(out=wf[:, l * P:(l + 1) * P], in_=WfT[l, :, :])
            nc.sync.dma_start(
                out=winT[:].rearrange("p (c q) -> p c q", c=FC),
                in_=WinT.ap().rearrange("(c p) q -> p c q", p=P))

            regs = {}
            for nb in set(NB_lo) | set(NB_hi):
                for c0 in range(0, nb, 8):
                    n_ = P * (min(c0 + 8, nb) - c0)
                    if n_ > 0 and n_ not in regs:
                        regs[n_] = nc.gpsimd.to_reg(n_)

            # ---- phase 0: Q0 = x @ Win.T + b_in (into hq) ----
            with tc.tile_pool(name="ph0", bufs=1) as p0:
                xall = p0.tile([P, FC * NLp], dtype16, tag="xall")
                nc.sync.dma_start(
                    out=xall[:].rearrange("p (c n) -> p c n", c=FC),
                    in_=xT.ap().rearrange("(c p) n -> p c n", p=P))
                for t in range(T):
                    q0p = psp.tile([P, P], f32, space="PSUM", tag="tp")
                    for c in range(FC):
                        nc.tensor.matmul(
                            out=q0p[:],
                            lhsT=xall[:, c * NLp + t * P: c * NLp + (t + 1) * P],
                            rhs=winT[:, c * P:(c + 1) * P],
                            start=(c == 0), stop=(c == FC - 1))
                    nc.vector.tensor_tensor(
                        out=hq[:, t * P:(t + 1) * P], in0=q0p[:],
                        in1=(binr[:] if t < T - 1 else binl[:]), op=OP.add)

            def seg_gather(dst3, dbase, src_ap, ixs, colbase, nb):
                """Gather nb 128-idx blocks into dst3[:, dbase:dbase+nb, :]."""
                for c0 in range(0, nb, 8):
                    c1 = min(c0 + 8, nb)
                    n_ = P * (c1 - c0)
                    nc.gpsimd.dma_gather(
                        dst3[:, dbase + c0:dbase + c1, :], src_ap,
                        ixs[:, (colbase + c0) * 8: (colbase + c1) * 8],
                        n_, regs[n_], 256)

            with tc.tile_pool(name="gat", bufs=3) as gp, \
                 tc.tile_pool(name="ohp", bufs=3) as op_:
                for li in range(KL):
                    # ---- dense phase ----
                    for t in range(T):
                        tp = psp.tile([P, P], f32, space="PSUM", tag="tp")
                        nc.tensor.transpose(
                            out=tp[:], in_=hq[:, t * P:(t + 1) * P],
                            identity=ident[:])
                        nc.scalar.activation(
                            out=QT[:, t * P:(t + 1) * P], in_=tp[:], func=AF.Copy)
                    for t in range(T):
                        qt = QT[:, t * P:(t + 1) * P]
                        p3 = psp.tile([P, P], f32, space="PSUM", tag="tp")
                        p4 = psp.tile([P, P], f32, space="PSUM", tag="p4")
                        pf = psp.tile([P, P], f32, space="PSUM", tag="pf")
                        nc.tensor.matmul(out=p3[:], lhsT=qt,
                                         rhs=w1[:, li * P:(li + 1) * P],
                                         start=True, stop=True)
                        nc.tensor.matmul(out=p4[:], lhsT=qt,
                                         rhs=w2[:, li * P:(li + 1) * P],
                                         start=True, stop=True)
                        nc.tensor.matmul(out=pf[:], lhsT=qt,
                                         rhs=wf[:, li * P:(li + 1) * P],
                                         start=True, stop=True)
                        gslice = uv[:, t * 256: t * 256 + P]
                        nc.scalar.activation(out=gslice, in_=p3[:], func=AF.Copy)
                        nc.scalar.activation(out=h4sb[:, t * P:(t + 1) * P],
                                             in_=p4[:], func=AF.Copy)
                        nc.vector.tensor_tensor(
                            out=uv[:, t * 256 + P: t * 256 + 256],
                            in0=gslice, in1=h4sb[:, t * P:(t + 1) * P], op=OP.add)
                        nc.scalar.activation(out=h1[:, t * P:(t + 1) * P],
                                             in_=pf[:], func=AF.Relu)
                    nc.sync.dma_start(
                        out=agin_v[:, :, :],
                        in_=uv[:].rearrange("p (t f) -> p t f", t=T))
                    nc.gpsimd.collective_compute(
                        "AllGather", OP.bypass,
                        replica_groups=[list(range(W))],
                        ins=[agin.ap()], outs=[table.ap()])

                    # ---- edge phase ----
                    olo = ohi = ob = 0
                    for t in range(T):
                        nb = NB_t[t]
                        g = gp.tile([P, NBmax * 256], dtype16, tag="g")
                        g3 = g[:].rearrange("p (b f) -> p b f", b=NBmax)
                        if NB_lo[t] > 0:
                            seg_gather(g3, 0, table.ap(), sxlo, olo, NB_lo[t])
                        if NB_hi[t] > 0:
                            seg_gather(g3, NB_lo[t],
                                       table.ap()[cfg.HIOFF:cfg.Ntab, :],
                                       sxhi, ohi, NB_hi[t])
                        o1 = op_.tile([P, NBmax * P], dtype16, tag="o1")
                        o2 = op_.tile([P, NBmax * P], dtype16, tag="o2")
                        nc.sync.dma_start(out=o1[:, :nb * P],
                                          in_=oh1[:, ob * P:(ob + nb) * P])
                        nc.sync.dma_start(out=o2[:, :nb * P],
                                          in_=oh2[:, ob * P:(ob + nb) * P])
                        aggp = ap_.tile([P, P], f32, space="PSUM", tag="agg")
                        for b in range(nb):
                            hsel = hp.tile([P, P], f32, space="PSUM", tag="hs")
                            nc.tensor.matmul(
                                out=hsel[:], lhsT=o1[:, b * P:(b + 1) * P],
                                rhs=h4sb[:, t * P:(t + 1) * P],
                                start=True, stop=True)
                            Gb = g[:, b * 256: b * 256 + P]
                            Wb = g[:, b * 256 + P: b * 256 + 256]
                            scr = mp.tile([P, P], dtype16, tag="scr")
                            scr2 = mp.tile([P, P], dtype16, tag="scr2")
                            d = mp.tile([P, 1], f32, tag="d")
                            nc.vector.tensor_tensor(
                                out=scr[:], in0=Gb, in1=hsel[:], op=OP.mult)
                            nc.scalar.activation(out=scr2[:], in_=scr[:],
                                                 func=AF.Copy, accum_out=d[:])
                            q = mp.tile([P, 1], f32, tag="q")
                            nc.scalar.activation(out=q[:], in_=d[:],
                                                 func=AF.Sigmoid)
                            m = mp.tile([P, P], dtype16, tag="m")
                            nc.vector.scalar_tensor_tensor(
                                out=m[:], in0=Wb, scalar=q[:], in1=Gb,
                                op0=OP.mult, op1=OP.subtract)
                            nc.tensor.matmul(
                                out=aggp[:], lhsT=o2[:, b * P:(b + 1) * P],
                                rhs=m[:], start=(b == 0), stop=(b == nb - 1))
                        nc.vector.tensor_tensor(
                            out=hq[:, t * P:(t + 1) * P],
                            in0=h1[:, t * P:(t + 1) * P], in1=aggp[:], op=OP.add)
                        olo += NB_lo[t]
                        ohi += NB_hi[t]
                        ob += nb

                    # ---- batched L2 normalize (hq in place) ----
                    nc.vector.tensor_tensor(out=sq[:], in0=hq[:], in1=hq[:],
                                            op=OP.mult)
                    nc.vector.tensor_reduce(
                        out=ss[:], in_=sq[:].rearrange("p (t f) -> p t f", t=T),
                        axis=AX.X, op=OP.add)
                    nc.scalar.activation(out=ss[:], in_=ss[:], func=AF.Sqrt)
                    nc.vector.tensor_scalar_max(out=ss[:], in0=ss[:],
                                                scalar1=1e-12)
                    nc.vector.reciprocal(out=rinv[:], in_=ss[:])
                    rinv3 = rinv[:].rearrange("p (t o) -> p t o", o=1)
                    nc.vector.tensor_tensor(
                        out=hq[:].rearrange("p (t f) -> p t f", t=T),
                        in0=hq[:].rearrange("p (t f) -> p t f", t=T),
                        in1=rinv3.to_broadcast([P, T, P]), op=OP.mult)

                # ---- phase 3: logits + log_softmax ----
                for t in range(T):
                    tp = psp.tile([P, P], f32, space="PSUM", tag="tp")
                    nc.tensor.transpose(out=tp[:], in_=hq[:, t * P:(t + 1) * P],
                                        identity=ident[:])
                    nc.scalar.activation(out=QT[:, t * P:(t + 1) * P],
                                         in_=tp[:], func=AF.Copy)
                    lp = psp.tile([P, C], f32, space="PSUM", tag="p4")
                    nc.tensor.matmul(out=lp[:], lhsT=QT[:, t * P:(t + 1) * P],
                                     rhs=wo[:], start=True, stop=True)
                    lg = mp.tile([P, C], f32, tag="lg")
                    nc.vector.tensor_tensor(out=lg[:], in0=lp[:], in1=boutr[:],
                                            op=OP.add)
                    mx = mp.tile([P, 1], f32, tag="mx")
                    nc.vector.tensor_reduce(out=mx[:], in_=lg[:], axis=AX.X,
                                            op=OP.max)
                    nmx = mp.tile([P, 1], f32, tag="nmx")
                    nc.vector.tensor_scalar_mul(out=nmx[:], in0=mx[:],
                                                scalar1=-1.0)
                    ex = mp.tile([P, C], f32, tag="ex")
                    se = mp.tile([P, 1], f32, tag="se")
                    nc.scalar.activation(out=ex[:], in_=lg[:], func=AF.Exp,
                                         bias=nmx[:], accum_out=se[:])
                    nc.scalar.activation(out=se[:], in_=se[:], func=AF.Ln)
                    nc.vector.tensor_tensor(out=mx[:], in0=mx[:], in1=se[:],
                                            op=OP.add)
                    nc.vector.tensor_scalar(
                        out=ostage[:, t * C:(t + 1) * C], in0=lg[:],
                        scalar1=mx[:], scalar2=None, op0=OP.subtract)
                nc.sync.dma_start(
                    out=out_v[:, :, :],
                    in_=ostage[:].rearrange("p (t c) -> p t c", t=T))

    nc.compile()
    return nc


def host_inputs(cfg, meta, per_core, inputs, np16=np.float16):
    x = np.asarray(inputs["x"], np.float32)
    W, NL = cfg.W, cfg.NL
    WinT = np.zeros((cfg.Fp, P), np16)
    WinT[:cfg.F] = np.asarray(inputs["W_in"], np.float32).T.astype(np16)
    W1T = np.ascontiguousarray(
        np.asarray(inputs["W1"], np.float32).transpose(0, 2, 1)).astype(np16)
    W2T = np.ascontiguousarray(
        np.asarray(inputs["W2"], np.float32).transpose(0, 2, 1)).astype(np16)
    WfT = np.ascontiguousarray(
        np.asarray(inputs["Wf"], np.float32).transpose(0, 2, 1)).astype(np16)
    WoT = np.ascontiguousarray(
        np.asarray(inputs["W_out"], np.float32).T).astype(np16)
    binr = np.tile(np.asarray(inputs["b_in"], np.float32)[None, :], (P, 1))
    boutr = np.tile(np.asarray(inputs["b_out"], np.float32)[None, :], (P, 1))
    maps = []
    for k in range(W):
        xk = np.zeros((cfg.Fp, cfg.NLp), np16)
        xk[:cfg.F, :NL] = x[k * NL:(k + 1) * NL].T.astype(np16)
        binl = binr.copy()
        binl[NL % P if NL % P else 0:] = 0.0   # pad-node rows of last tile
        m = dict(
            xT=xk, WinT=WinT, W1T=W1T, W2T=W2T, WfT=WfT, WoutT=WoT,
            b_in_rep=np.ascontiguousarray(binr, np.float32),
            b_in_lst=np.ascontiguousarray(binl, np.float32),
            b_out_rep=np.ascontiguousarray(boutr, np.float32),
            ixlo=per_core[k]["ixlo"], ixhi=per_core[k]["ixhi"],
            oh1=per_core[k]["oh1"], oh2=per_core[k]["oh2"])
        maps.append(m)
    return maps


def _install_profile_hook():
    """Provide antenv.axon_hooks (absent in this image) so that
    run_bass_kernel_spmd(trace=True) can collect an NTFF profile."""
    try:
        import types
        import antenv
        if "antenv.axon_hooks" not in sys.modules:
            mod = types.ModuleType("antenv.axon_hooks")
            state = {"hook": None}
            mod.set_axon_ntff_profile_hook = lambda h: state.__setitem__("hook", h)
            mod.get_axon_ntff_profile_hook = lambda: state["hook"]
            sys.modules["antenv.axon_hooks"] = mod
            antenv.axon_hooks = mod
        from antenv.axon_hooks import (get_axon_ntff_profile_hook,
                                       set_axon_ntff_profile_hook)
        if get_axon_ntff_profile_hook() is None:
            from trn_agent_boot.trn_boot import _ntff_profile_via_ctypes
            set_axon_ntff_profile_hook(
                _ntff_profile_via_ctypes("/opt/axon/libaxon_pjrt.so"))
        return True
    except Exception as e:  # degrade to untraced run
        print(f"profile hook unavailable: {e}")
        return False


def kernel(**inputs):
    cfg = Cfg()
    edge_index = np.asarray(inputs["edge_index"])
    meta, per_core = plan(cfg, edge_index)
    nc = build(cfg, meta)
    in_maps = host_inputs(cfg, meta, per_core, inputs)
    trace = bool(int(os.environ.get("GNN_TRACE", "0")))
    if trace:
        trace = _install_profile_hook()
    from concourse import bass_utils
    res = bass_utils.run_bass_kernel_spmd(
        nc, in_maps, core_ids=list(range(cfg.W)), trace=trace)
    if res.exec_time_ns is not None:
        print(f"HW exec time: {res.exec_time_ns} ns")
    outs = [res.results[k]["out"][:cfg.NL] for k in range(cfg.W)]
    return np.concatenate(outs, axis=0).astype(np.float32)
